# revision 34
# baseline (speedup 1.0000x reference)
"""Multi-head attention Trainium2 kernel (B=4, S=2048, D=1024, H=16, A=64).

Sharding: 8 cores = batch (4) x head-half (2). Core i handles batch i//2,
heads (i%2)*8 .. (i%2)*8+8. No collectives; host assembles output.

Design (HW exec ~343us vs 453us hi/lo-f32r baseline; rel err ~5e-3):
  - All inputs arrive host-pretransposed [D, S] in fp16 (PE streams 16-bit
    moving operands at 2.4 GHz = 2x the f32r rate; fp16 keeps ~1e-3 accuracy
    where bf16 would cost ~1e-2). Halves input DMA vs hi/lo bf16 pairs.
  - Projections fp16 -> qhT/khT stored fp16, vh stored bf16 (vh/wt matmul
    runs bf16 because exp outputs need bf16's exponent range).
  - Scores: per head-pair concurrent K=64 fp16 matmul pairs, tile_position
    (0,0)/(64,0), into one [128,1024] PSUM tile.
  - Exp split 9:7 across engines per sk-16 group: ACT (exact spline exp,
    bf16 out, (N+352)/1.2 ns) and DVE via Schraudolph bit-trick:
    i16 = round(s*128/ln2 + 16248); bitcast bf16  (~1.5% mean rel err,
    fine post-softmax; DVE f32-in rate ~110G el/s, similar to ACT, but the
    two engines run in parallel so exp never binds).
  - attn' [65, Sq] accumulated in PSUM over sk (row 64 = softmax denom via
    ones-column in vh, set once by memset); copied [65,512] to SBUF (P0 on
    ACT, P1 on DVE to free PSUM promptly) and DMA'd out untransposed. Host
    does divide-by-denominator, +bv, and the transpose (free).
  - PSUM budget (8 banks x 2KB): sc [128,1024]x2bufs = 4, att [65,512]x2 =
    2, pp (proj) [128,512]x2 = 2.
  - Pipelining: upfront only k-proj pair0 + q-proj np0-h0 (~20us incl DMA,
    after ~6.5us fixed preamble + ~0.7us/descriptor issue on sync+scalar);
    iteration (0,0) runs scores/exp-only for 16 slots (LAG=16, wt ring 16)
    so attn never blocks on v's DMA; ALL v-proj and the other pairs' k/q
    projections are injected just-in-time into phase-B per-sk slack. PE is
    ~100% busy from ~25us to the end (PE is the global bottleneck: ~310us
    busy = ~246us 16-bit stream + LDW/fill exposure).
"""

import sys

sys.path.insert(0, "/opt/trn_rl_repo")

import numpy as np

B, S, D = 4, 2048, 1024
H, A = 16, 64
NCORES = 8
HL = H // 2          # heads per core
NPAIR = HL // 2      # head pairs per core
ND = D // 128        # D chunks
NP2 = 2              # S chunks of 1024
NSQ = S // 512       # Sq chunks for phase B
NSK = S // 128       # Sk tiles
AC = A + 1           # vh columns incl. ones column

A16 = 128.0 / float(np.log(2.0))   # Schraudolph scale for bf16 bit pattern
B16 = float(127 * 128 - 8)         # exponent bias minus calibrated C=8
ACT_SKS = (0, 1, 2, 4, 6, 8, 10, 12, 14)  # 9 ACT : 7 DVE per 16


def _build():
    import concourse.tile as tile
    from concourse import bacc, mybir

    F32 = mybir.dt.float32
    F16 = mybir.dt.float16
    BF16 = mybir.dt.bfloat16
    I16 = mybir.dt.int16
    ADD = mybir.AluOpType.add
    MUL = mybir.AluOpType.mult
    EXP = mybir.ActivationFunctionType.Exp

    nc = bacc.Bacc("TRN2")

    x_d = {}
    for x in ("q", "k", "v"):
        x_d[x] = nc.dram_tensor(f"x{x}", [D, S], F16, kind="ExternalInput").ap()
    wq_d = nc.dram_tensor("wq", [D, HL * A], F16, kind="ExternalInput").ap()
    wk_d = nc.dram_tensor("wk", [D, HL * A], F16, kind="ExternalInput").ap()
    wv_d = nc.dram_tensor("wv", [D, HL * A], F16, kind="ExternalInput").ap()
    bqk_d = nc.dram_tensor("bqk", [128, 2 * NPAIR], F32, kind="ExternalInput").ap()
    # out blocks [h, sq, a-row(65), q(512)]; host divides by row 64 + transposes
    out_d = nc.dram_tensor("out", [HL, NSQ, AC, 512], F32, kind="ExternalOutput").ap()

    with tile.TileContext(nc) as tc:
        with (
            tc.tile_pool(name="consts", bufs=1) as consts,
            tc.tile_pool(name="persist", bufs=1) as persist,
            tc.tile_pool(name="work", bufs=1) as work,
            tc.tile_pool(name="ps", bufs=1, space="PSUM") as ps,
        ):
            bqk_sb = consts.tile([128, 2 * NPAIR], F32, tag="bqk")
            bq_sb = bqk_sb[:, 0:NPAIR]
            bk_sb = bqk_sb[:, NPAIR : 2 * NPAIR]

            # weights, p-major layout [128, d-chunk, cols]; DMA issue order
            # below is criticality order (wk first, wq/wv interleaved with x);
            # everything split into halves so the critical set spreads across
            # parallel DMA queues
            HD = ND // 2

            def load_w(w_sb, w_d, e0=None, e1=None):
                src = w_d.rearrange("(c p) n -> p c n", p=128)
                (e0 or nc.sync).dma_start(w_sb[:, 0:HD, :], src[:, 0:HD, :])
                (e1 or nc.sync).dma_start(w_sb[:, HD:ND, :], src[:, HD:ND, :])

            wk_sb = work.tile([128, ND, HL * A], F16, tag="wk", name="wk_sb")
            wq_sb = work.tile([128, ND, HL * A], F16, tag="wq", name="wq_sb")
            wv_sb = work.tile([128, ND, HL * A], F16, tag="wv", name="wv_sb")

            qhT = [
                persist.tile([128, S], F16, tag=f"qhT{p}", name=f"qhT{p}")
                for p in range(NPAIR)
            ]
            khT = [
                persist.tile([128, S], F16, tag=f"khT{p}", name=f"khT{p}")
                for p in range(NPAIR)
            ]
            vh = persist.tile([128, HL, NSK, AC], BF16, tag="vh")
            # softmax-denominator ones column, set once
            nc.gpsimd.memset(vh[:, :, :, A : A + 1], 1.0)

            # ---- input tiles: all resident (96KB/partition); plain per-
            # d-chunk [128,1024] DMAs (2KB lines, fastest observed), issued
            # in deadline order for the (0,0) lag-16 pipeline ----
            xTn = {}

            def load_x(x, np_, engines=None):
                tiles = []
                for d in range(ND):
                    t = persist.tile(
                        [128, 1024], F16, tag=f"{x}T{np_}_{d}",
                        name=f"{x}T{np_}_{d}",
                    )
                    eng = engines[d % len(engines)] if engines else nc.sync
                    eng.dma_start(
                        t,
                        x_d[x][
                            d * 128 : (d + 1) * 128,
                            np_ * 1024 : (np_ + 1) * 1024,
                        ],
                    )
                    tiles.append(t)
                xTn[(x, np_)] = tiles

            # Only 8 DMA semaphores exist; descriptor N+8 blocks its queue
            # until transfer N completes. Let the 8 critical k-np0 tiles grab
            # all 8 slots first; weights/biases/queries follow in need order.
            load_w(wk_sb, wk_d, nc.sync, nc.scalar)     # slots 1-2
            nc.sync.dma_start(bqk_sb, bqk_d)            # slot 3 (32B lines)
            load_x("k", 0, engines=[nc.sync, nc.scalar])   # scores sk0-7
            load_w(wq_sb, wq_d, nc.sync, nc.scalar)
            load_x("q", 0, engines=[nc.scalar, nc.sync])    # sq0/sq1 queries
            load_x("k", 1)             # scores sk8-15
            load_w(wv_sb, wv_d)
            load_x("v", 0)             # vh m0-7 (attn from slot 16)
            load_x("v", 1)             # vh m8-15
            load_x("q", 1)

            # ---- projection piece emitters ----
            def proj_piece(x, np_, p, half):
                w_sb = wk_sb if x == "k" else wq_sb
                bias_sb = bk_sb if x == "k" else bq_sb
                xhT = khT if x == "k" else qhT

                def emit():
                    pp = ps.tile([128, 512], F32, tag="pp", name="pp", bufs=2)
                    for d in range(ND):
                        nc.tensor.matmul(
                            pp,
                            w_sb[:, d, p * 128 : (p + 1) * 128],
                            xTn[(x, np_)][d][:, half * 512 : (half + 1) * 512],
                            start=(d == 0),
                            stop=(d == ND - 1),
                        )
                    col = np_ * 1024 + half * 512
                    nc.vector.tensor_scalar(
                        xhT[p][:, col : col + 512], pp, bias_sb[:, p : p + 1],
                        None, ADD,
                    )
                return emit

            def vproj_piece(m):
                np_, t = divmod(m, 8)

                def emit():
                    pv = ps.tile([128, HL * A], F32, tag="pp", name="pv", bufs=2)
                    for d in range(ND):
                        nc.tensor.matmul(
                            pv,
                            xTn[("v", np_)][d][:, t * 128 : (t + 1) * 128],
                            wv_sb[:, d, :],
                            start=(d == 0),
                            stop=(d == ND - 1),
                        )
                    nc.vector.tensor_copy(
                        vh[:, :, m, 0:A],
                        pv.rearrange("p (h c) -> p h c", h=HL),
                    )
                return emit

            # ---- upfront phase A: pair-0 k np0 + q np0 h0 (first scores /
            # sq0); everything else just-in-time (stream order = readiness)
            proj_piece("k", 0, 0, 0)()
            proj_piece("k", 0, 0, 1)()
            proj_piece("q", 0, 0, 0)()

            # ---- injection schedule: (p, sq) -> list of (sk, emitter) ----
            sched = {}

            def put(p, sq, sk, em):
                sched.setdefault((p, sq), []).append((sk, em))

            # (0,0) runs with attn lag 16: slots 0-15 are scores/exp only
            # (pair-0 np1 k pieces injected before their consuming scores),
            # v-proj pieces ride alongside the lagged attn from slot 16
            for m in range(NSK):
                put(0, 0, 16 + m, vproj_piece(m))
            put(0, 0, 1, proj_piece("q", 0, 0, 1))   # (0,1) queries
            put(0, 0, 4, proj_piece("k", 1, 0, 0))   # scores sk8+
            put(0, 0, 9, proj_piece("k", 1, 0, 1))   # scores sk12+
            # q np1 p0 during (0,1)
            put(0, 1, 3, proj_piece("q", 1, 0, 0))
            put(0, 1, 9, proj_piece("q", 1, 0, 1))
            # pair p_ k/q injected during earlier iterations
            for p_ in (1, 2, 3):
                base = 4 * (p_ - 1)  # iterations it2,it3 / it6,it7 / it10,it11
                it2 = divmod(base + 2, 4)
                it3 = divmod(base + 3, 4)
                put(*it2, 2, proj_piece("k", 0, p_, 0))
                put(*it2, 8, proj_piece("k", 0, p_, 1))
                put(*it3, 2, proj_piece("k", 1, p_, 0))
                put(*it3, 8, proj_piece("k", 1, p_, 1))
                put(*it3, 5, proj_piece("q", 0, p_, 0))
                put(*it3, 11, proj_piece("q", 0, p_, 1))
                it5 = divmod(4 * p_ + 1, 4)
                put(*it5, 3, proj_piece("q", 1, p_, 0))
                put(*it5, 9, proj_piece("q", 1, p_, 1))

            # ---------------- Phase B: attention ----------------
            for p in range(NPAIR):
                h0, h1 = 2 * p, 2 * p + 1
                for sq in range(NSQ):
                    inj = dict()
                    for sk, em in sched.get((p, sq), []):
                        inj.setdefault(sk, []).append(em)
                    P0 = ps.tile([65, 512], F32, tag="att", name="P0", bufs=2)
                    P1 = ps.tile([65, 512], F32, tag="att", name="P1", bufs=2)
                    LAG = 16 if (p, sq) == (0, 0) else 2
                    wts = [None] * NSK
                    for sk in range(NSK + LAG):
                        if sk < NSK:
                            Sc = ps.tile(
                                [128, 1024], F32, tag="sc", name="Sc", bufs=2
                            )
                            nc.tensor.matmul(
                                Sc[:, 0:512],
                                khT[p][0:64, sk * 128 : (sk + 1) * 128],
                                qhT[p][0:64, sq * 512 : (sq + 1) * 512],
                                start=True, stop=True, tile_position=(0, 0),
                            )
                            nc.tensor.matmul(
                                Sc[:, 512:1024],
                                khT[p][64:128, sk * 128 : (sk + 1) * 128],
                                qhT[p][64:128, sq * 512 : (sq + 1) * 512],
                                start=True, stop=True, tile_position=(64, 0),
                            )
                            if sk in ACT_SKS:
                                wt = work.tile(
                                    [128, 1024], BF16, tag="wt", name="wt", bufs=16
                                )
                                nc.scalar.activation(wt, Sc, EXP)
                            else:
                                wti = work.tile(
                                    [128, 1024], I16, tag="wt", name="wti", bufs=16
                                )
                                nc.vector.tensor_scalar(
                                    wti, Sc, A16, B16, MUL, ADD
                                )
                                wt = wti.bitcast(BF16)
                            wts[sk] = wt
                        for em in inj.get(sk, []):
                            em()
                        if sk >= LAG:
                            k0 = sk - LAG
                            st = k0 == 0
                            sp = k0 == NSK - 1
                            nc.tensor.matmul(
                                P0, vh[:, h0, k0, :], wts[k0][:, 0:512],
                                start=st, stop=sp,
                            )
                            nc.tensor.matmul(
                                P1, vh[:, h1, k0, :], wts[k0][:, 512:1024],
                                start=st, stop=sp,
                            )
                    # split the PSUM-freeing copies across ACT and DVE so the
                    # next-next iteration's P0/P1 slots free promptly
                    att_s0 = work.tile(
                        [65, 512], F32, tag="atts", name="att_s0", bufs=2
                    )
                    nc.scalar.copy(att_s0, P0)
                    nc.sync.dma_start(out_d[h0, sq], att_s0)
                    att_s1 = work.tile(
                        [65, 512], F32, tag="atts", name="att_s1", bufs=2
                    )
                    nc.vector.tensor_copy(att_s1, P1)
                    nc.sync.dma_start(out_d[h1, sq], att_s1)

    nc.compile()
    return nc


_NC_CACHE = None
_LAST_IN_MAPS = None


def kernel(**inputs: np.ndarray) -> np.ndarray:
    global _NC_CACHE, _LAST_IN_MAPS

    from concourse.bass_utils import run_bass_kernel_spmd

    q = np.ascontiguousarray(inputs["q"], dtype=np.float32)
    k = np.ascontiguousarray(inputs["k"], dtype=np.float32)
    v = np.ascontiguousarray(inputs["v"], dtype=np.float32)
    Wq = np.asarray(inputs["Wq"], dtype=np.float32)
    Wk = np.asarray(inputs["Wk"], dtype=np.float32)
    Wv = np.asarray(inputs["Wv"], dtype=np.float32)
    bq = np.asarray(inputs["bq"], dtype=np.float32)
    bk = np.asarray(inputs["bk"], dtype=np.float32)
    bv = np.asarray(inputs["bv"], dtype=np.float32)

    if _NC_CACHE is None:
        _NC_CACHE = _build()
    nc = _NC_CACHE

    def xt16(x):
        return np.ascontiguousarray(x.T).astype(np.float16)

    def pack_w(W, g):
        # [H,D,A] slice -> [D, HL*A], heads side by side
        return np.ascontiguousarray(
            W[g * HL : (g + 1) * HL].transpose(1, 0, 2).reshape(D, HL * A)
        ).astype(np.float16)

    def pack_b(bvec, g):
        # [H,A] slice -> [128, NPAIR]: column p = concat(b[2p], b[2p+1])
        bg = bvec[g * HL : (g + 1) * HL]
        return np.ascontiguousarray(bg.reshape(NPAIR, 128).T)

    xq = [xt16(q[b_]) for b_ in range(B)]
    xk = [xt16(k[b_]) for b_ in range(B)]
    xv = [xt16(v[b_]) for b_ in range(B)]

    in_maps = []
    for i in range(NCORES):
        b_, g = i // 2, i % 2
        in_maps.append(
            {
                "xq": xq[b_],
                "xk": xk[b_],
                "xv": xv[b_],
                "wq": pack_w(Wq, g),
                "wk": pack_w(Wk, g),
                "wv": pack_w(Wv, g),
                "bqk": np.ascontiguousarray(
                    np.concatenate([pack_b(bq, g), pack_b(bk, g)], axis=1)
                ),
            }
        )

    _LAST_IN_MAPS = in_maps
    res = run_bass_kernel_spmd(nc, in_maps, core_ids=list(range(NCORES)))

    out = np.empty((B, S, H * A), dtype=np.float32)
    for i in range(NCORES):
        b_, g = i // 2, i % 2
        blk = res.results[i]["out"]  # [HL, NSQ, 65, 512]
        o = blk[:, :, :A, :] / blk[:, :, A : A + 1, :]  # divide by denom row
        o = o + bv[g * HL : (g + 1) * HL][:, None, :, None]  # bias post-divide
        # [HL, NSQ, A, 512] -> [S, HL*A]
        out[b_, :, g * HL * A : (g + 1) * HL * A] = (
            o.transpose(1, 3, 0, 2).reshape(S, HL * A)
        )
    return out


# revision 35
# speedup vs baseline: 1.1865x; 1.1865x over previous
"""Multi-head attention Trainium2 kernel (B=4, S=2048, D=1024, H=16, A=64).

Sharding: 8 cores = batch (4) x head-half (2). Core i handles batch i//2,
heads (i%2)*8 .. (i%2)*8+8. No collectives; host assembles output.

Design (HW exec ~343us vs 453us hi/lo-f32r baseline; rel err ~5e-3):
  - All inputs arrive host-pretransposed [D, S] in fp16 (PE streams 16-bit
    moving operands at 2.4 GHz = 2x the f32r rate; fp16 keeps ~1e-3 accuracy
    where bf16 would cost ~1e-2). Halves input DMA vs hi/lo bf16 pairs.
  - Projections fp16 -> qhT/khT stored fp16, vh stored bf16 (vh/wt matmul
    runs bf16 because exp outputs need bf16's exponent range).
  - Scores: per head-pair concurrent K=64 fp16 matmul pairs, tile_position
    (0,0)/(64,0), into one [128,1024] PSUM tile.
  - Exp split 9:7 across engines per sk-16 group: ACT (exact spline exp,
    bf16 out, (N+352)/1.2 ns) and DVE via Schraudolph bit-trick:
    i16 = round(s*128/ln2 + 16248); bitcast bf16  (~1.5% mean rel err,
    fine post-softmax; DVE f32-in rate ~110G el/s, similar to ACT, but the
    two engines run in parallel so exp never binds).
  - attn' [65, Sq] accumulated in PSUM over sk (row 64 = softmax denom via
    ones-column in vh, set once by memset); copied [65,512] to SBUF (P0 on
    ACT, P1 on DVE to free PSUM promptly) and DMA'd out untransposed. Host
    does divide-by-denominator, +bv, and the transpose (free).
  - PSUM budget (8 banks x 2KB): sc [128,1024]x2bufs = 4, att [65,512]x2 =
    2, pp (proj) [128,512]x2 = 2.
  - Pipelining: upfront only k-proj pair0 + q-proj np0-h0 (~20us incl DMA,
    after ~6.5us fixed preamble + ~0.7us/descriptor issue on sync+scalar);
    iteration (0,0) runs scores/exp-only for 16 slots (LAG=16, wt ring 16)
    so attn never blocks on v's DMA; ALL v-proj and the other pairs' k/q
    projections are injected just-in-time into phase-B per-sk slack. PE is
    ~100% busy from ~25us to the end (PE is the global bottleneck: ~310us
    busy = ~246us 16-bit stream + LDW/fill exposure).
"""

import sys

sys.path.insert(0, "/opt/trn_rl_repo")

import numpy as np

B, S, D = 4, 2048, 1024
H, A = 16, 64
NCORES = 8
HL = H // 2          # heads per core
NPAIR = HL // 2      # head pairs per core
ND = D // 128        # D chunks
NP2 = 2              # S chunks of 1024
NSQ = S // 512       # Sq chunks for phase B
NSK = S // 128       # Sk tiles
AC = A + 1           # vh columns incl. ones column

A16 = 128.0 / float(np.log(2.0))   # Schraudolph scale for bf16 bit pattern
B16 = float(127 * 128 - 8)         # exponent bias minus calibrated C=8
ACT_SKS = (0, 1, 2, 4, 6, 8, 10, 12, 14)  # 9 ACT : 7 DVE per 16


def _build():
    import concourse.tile as tile
    from concourse import bacc, mybir

    F32 = mybir.dt.float32
    F16 = mybir.dt.float16
    BF16 = mybir.dt.bfloat16
    I16 = mybir.dt.int16
    ADD = mybir.AluOpType.add
    MUL = mybir.AluOpType.mult
    EXP = mybir.ActivationFunctionType.Exp

    nc = bacc.Bacc("TRN2")

    x_d = {}
    for x in ("q", "k", "v"):
        x_d[x] = nc.dram_tensor(f"x{x}", [D, S], F16, kind="ExternalInput").ap()
    wq_d = nc.dram_tensor("wq", [D, HL * A], F16, kind="ExternalInput").ap()
    wk_d = nc.dram_tensor("wk", [D, HL * A], F16, kind="ExternalInput").ap()
    wv_d = nc.dram_tensor("wv", [D, HL * A], F16, kind="ExternalInput").ap()
    bq_d = nc.dram_tensor("bq", [128, NPAIR], F32, kind="ExternalInput").ap()
    bk_d = nc.dram_tensor("bk", [128, NPAIR], F32, kind="ExternalInput").ap()
    # out blocks [h, sq, a-row(65), q(512)]; host divides by row 64 + transposes
    out_d = nc.dram_tensor("out", [HL, NSQ, AC, 512], F32, kind="ExternalOutput").ap()

    with tile.TileContext(nc) as tc:
        with (
            tc.tile_pool(name="consts", bufs=1) as consts,
            tc.tile_pool(name="persist", bufs=1) as persist,
            tc.tile_pool(name="work", bufs=1) as work,
            tc.tile_pool(name="ps", bufs=1, space="PSUM") as ps,
        ):
            bq_sb = consts.tile([128, NPAIR], F32, tag="bq")
            bk_sb = consts.tile([128, NPAIR], F32, tag="bk")

            # weights, p-major layout [128, d-chunk, cols]; DMA issue order
            # below is criticality order (wk first, wq/wv interleaved with x);
            # everything split into halves so the critical set spreads across
            # parallel DMA queues
            HD = ND // 2

            def load_w(w_sb, w_d, e0=None, e1=None):
                src = w_d.rearrange("(c p) n -> p c n", p=128)
                (e0 or nc.sync).dma_start(w_sb[:, 0:HD, :], src[:, 0:HD, :])
                (e1 or nc.sync).dma_start(w_sb[:, HD:ND, :], src[:, HD:ND, :])

            wk_sb = work.tile([128, ND, HL * A], F16, tag="wk", name="wk_sb")
            wq_sb = work.tile([128, ND, HL * A], F16, tag="wq", name="wq_sb")
            wv_sb = work.tile([128, ND, HL * A], F16, tag="wv", name="wv_sb")

            qhT = [
                persist.tile([128, S], F16, tag=f"qhT{p}", name=f"qhT{p}")
                for p in range(NPAIR)
            ]
            khT = [
                persist.tile([128, S], F16, tag=f"khT{p}", name=f"khT{p}")
                for p in range(NPAIR)
            ]
            vh = persist.tile([128, HL, NSK, AC], BF16, tag="vh")
            # softmax-denominator ones column, set once
            nc.gpsimd.memset(vh[:, :, :, A : A + 1], 1.0)

            # ---- input tiles: all resident (96KB/partition); plain per-
            # d-chunk [128,1024] DMAs (2KB lines, fastest observed), issued
            # in deadline order for the (0,0) lag-16 pipeline ----
            xTn = {}

            def load_x(x, np_, engines=None):
                tiles = []
                for d in range(ND):
                    t = persist.tile(
                        [128, 1024], F16, tag=f"{x}T{np_}_{d}",
                        name=f"{x}T{np_}_{d}",
                    )
                    eng = engines[d % len(engines)] if engines else nc.sync
                    eng.dma_start(
                        t,
                        x_d[x][
                            d * 128 : (d + 1) * 128,
                            np_ * 1024 : (np_ + 1) * 1024,
                        ],
                    )
                    tiles.append(t)
                xTn[(x, np_)] = tiles

            # Only 8 DMA semaphores exist; descriptor N+8 blocks its queue
            # until transfer N completes. Let the 8 critical k-np0 tiles grab
            # all 8 slots first; weights/biases/queries follow in need order.
            load_x("k", 0, engines=[nc.sync, nc.scalar])   # scores sk0-7
            load_w(wk_sb, wk_d, nc.sync, nc.scalar)
            nc.sync.dma_start(bq_sb, bq_d)     # slow 16B-line transfers,
            nc.scalar.dma_start(bk_sb, bk_d)   # needed by bias-adds ~16us
            load_w(wq_sb, wq_d, nc.sync, nc.scalar)
            load_x("q", 0, engines=[nc.scalar, nc.sync])    # sq0/sq1 queries
            load_x("k", 1)             # scores sk8-15
            load_w(wv_sb, wv_d)
            load_x("v", 0)             # vh m0-7 (attn from slot 16)
            load_x("v", 1)             # vh m8-15
            load_x("q", 1)

            # ---- projection piece emitters ----
            def proj_piece(x, np_, p, half):
                w_sb = wk_sb if x == "k" else wq_sb
                bias_sb = bk_sb if x == "k" else bq_sb
                xhT = khT if x == "k" else qhT

                def emit():
                    pp = ps.tile([128, 512], F32, tag="pp", name="pp", bufs=2)
                    for d in range(ND):
                        nc.tensor.matmul(
                            pp,
                            w_sb[:, d, p * 128 : (p + 1) * 128],
                            xTn[(x, np_)][d][:, half * 512 : (half + 1) * 512],
                            start=(d == 0),
                            stop=(d == ND - 1),
                        )
                    col = np_ * 1024 + half * 512
                    nc.vector.tensor_scalar(
                        xhT[p][:, col : col + 512], pp, bias_sb[:, p : p + 1],
                        None, ADD,
                    )
                return emit

            def vproj_piece(m):
                np_, t = divmod(m, 8)

                def emit():
                    pv = ps.tile([128, HL * A], F32, tag="pp", name="pv", bufs=2)
                    for d in range(ND):
                        nc.tensor.matmul(
                            pv,
                            xTn[("v", np_)][d][:, t * 128 : (t + 1) * 128],
                            wv_sb[:, d, :],
                            start=(d == 0),
                            stop=(d == ND - 1),
                        )
                    nc.vector.tensor_copy(
                        vh[:, :, m, 0:A],
                        pv.rearrange("p (h c) -> p h c", h=HL),
                    )
                return emit

            # ---- upfront phase A: pair-0 k np0 + q np0 h0 (first scores /
            # sq0); everything else just-in-time (stream order = readiness)
            proj_piece("k", 0, 0, 0)()
            proj_piece("k", 0, 0, 1)()
            proj_piece("q", 0, 0, 0)()

            # ---- injection schedule: (p, sq) -> list of (sk, emitter) ----
            sched = {}

            def put(p, sq, sk, em):
                sched.setdefault((p, sq), []).append((sk, em))

            # (0,0) runs with attn lag 16: slots 0-15 are scores/exp only
            # (pair-0 np1 k pieces injected before their consuming scores),
            # v-proj pieces ride alongside the lagged attn from slot 16
            for m in range(NSK):
                put(0, 0, 16 + m, vproj_piece(m))
            put(0, 0, 1, proj_piece("q", 0, 0, 1))   # (0,1) queries
            put(0, 0, 4, proj_piece("k", 1, 0, 0))   # scores sk8+
            put(0, 0, 9, proj_piece("k", 1, 0, 1))   # scores sk12+
            # q np1 p0 during (0,1)
            put(0, 1, 3, proj_piece("q", 1, 0, 0))
            put(0, 1, 9, proj_piece("q", 1, 0, 1))
            # pair p_ k/q injected during earlier iterations
            for p_ in (1, 2, 3):
                base = 4 * (p_ - 1)  # iterations it2,it3 / it6,it7 / it10,it11
                it2 = divmod(base + 2, 4)
                it3 = divmod(base + 3, 4)
                put(*it2, 2, proj_piece("k", 0, p_, 0))
                put(*it2, 8, proj_piece("k", 0, p_, 1))
                put(*it3, 2, proj_piece("k", 1, p_, 0))
                put(*it3, 8, proj_piece("k", 1, p_, 1))
                put(*it3, 5, proj_piece("q", 0, p_, 0))
                put(*it3, 11, proj_piece("q", 0, p_, 1))
                it5 = divmod(4 * p_ + 1, 4)
                put(*it5, 3, proj_piece("q", 1, p_, 0))
                put(*it5, 9, proj_piece("q", 1, p_, 1))

            # ---------------- Phase B: attention ----------------
            for p in range(NPAIR):
                h0, h1 = 2 * p, 2 * p + 1
                for sq in range(NSQ):
                    inj = dict()
                    for sk, em in sched.get((p, sq), []):
                        inj.setdefault(sk, []).append(em)
                    P0 = ps.tile([65, 512], F32, tag="att", name="P0", bufs=2)
                    P1 = ps.tile([65, 512], F32, tag="att", name="P1", bufs=2)
                    LAG = 16 if (p, sq) == (0, 0) else 2
                    wts = [None] * NSK
                    for sk in range(NSK + LAG):
                        if sk < NSK:
                            Sc = ps.tile(
                                [128, 1024], F32, tag="sc", name="Sc", bufs=2
                            )
                            nc.tensor.matmul(
                                Sc[:, 0:512],
                                khT[p][0:64, sk * 128 : (sk + 1) * 128],
                                qhT[p][0:64, sq * 512 : (sq + 1) * 512],
                                start=True, stop=True, tile_position=(0, 0),
                            )
                            nc.tensor.matmul(
                                Sc[:, 512:1024],
                                khT[p][64:128, sk * 128 : (sk + 1) * 128],
                                qhT[p][64:128, sq * 512 : (sq + 1) * 512],
                                start=True, stop=True, tile_position=(64, 0),
                            )
                            if sk in ACT_SKS:
                                wt = work.tile(
                                    [128, 1024], BF16, tag="wt", name="wt", bufs=16
                                )
                                nc.scalar.activation(wt, Sc, EXP)
                            else:
                                wti = work.tile(
                                    [128, 1024], I16, tag="wt", name="wti", bufs=16
                                )
                                nc.vector.tensor_scalar(
                                    wti, Sc, A16, B16, MUL, ADD
                                )
                                wt = wti.bitcast(BF16)
                            wts[sk] = wt
                        for em in inj.get(sk, []):
                            em()
                        if sk >= LAG:
                            k0 = sk - LAG
                            st = k0 == 0
                            sp = k0 == NSK - 1
                            nc.tensor.matmul(
                                P0, vh[:, h0, k0, :], wts[k0][:, 0:512],
                                start=st, stop=sp,
                            )
                            nc.tensor.matmul(
                                P1, vh[:, h1, k0, :], wts[k0][:, 512:1024],
                                start=st, stop=sp,
                            )
                    # split the PSUM-freeing copies across ACT and DVE so the
                    # next-next iteration's P0/P1 slots free promptly
                    att_s0 = work.tile(
                        [65, 512], F32, tag="atts", name="att_s0", bufs=2
                    )
                    nc.scalar.copy(att_s0, P0)
                    nc.sync.dma_start(out_d[h0, sq], att_s0)
                    att_s1 = work.tile(
                        [65, 512], F32, tag="atts", name="att_s1", bufs=2
                    )
                    nc.vector.tensor_copy(att_s1, P1)
                    nc.sync.dma_start(out_d[h1, sq], att_s1)

    nc.compile()
    return nc


_NC_CACHE = None
_LAST_IN_MAPS = None


def kernel(**inputs: np.ndarray) -> np.ndarray:
    global _NC_CACHE, _LAST_IN_MAPS

    from concourse.bass_utils import run_bass_kernel_spmd

    q = np.ascontiguousarray(inputs["q"], dtype=np.float32)
    k = np.ascontiguousarray(inputs["k"], dtype=np.float32)
    v = np.ascontiguousarray(inputs["v"], dtype=np.float32)
    Wq = np.asarray(inputs["Wq"], dtype=np.float32)
    Wk = np.asarray(inputs["Wk"], dtype=np.float32)
    Wv = np.asarray(inputs["Wv"], dtype=np.float32)
    bq = np.asarray(inputs["bq"], dtype=np.float32)
    bk = np.asarray(inputs["bk"], dtype=np.float32)
    bv = np.asarray(inputs["bv"], dtype=np.float32)

    if _NC_CACHE is None:
        _NC_CACHE = _build()
    nc = _NC_CACHE

    def xt16(x):
        return np.ascontiguousarray(x.T).astype(np.float16)

    def pack_w(W, g):
        # [H,D,A] slice -> [D, HL*A], heads side by side
        return np.ascontiguousarray(
            W[g * HL : (g + 1) * HL].transpose(1, 0, 2).reshape(D, HL * A)
        ).astype(np.float16)

    def pack_b(bvec, g):
        # [H,A] slice -> [128, NPAIR]: column p = concat(b[2p], b[2p+1])
        bg = bvec[g * HL : (g + 1) * HL]
        return np.ascontiguousarray(bg.reshape(NPAIR, 128).T)

    xq = [xt16(q[b_]) for b_ in range(B)]
    xk = [xt16(k[b_]) for b_ in range(B)]
    xv = [xt16(v[b_]) for b_ in range(B)]

    in_maps = []
    for i in range(NCORES):
        b_, g = i // 2, i % 2
        in_maps.append(
            {
                "xq": xq[b_],
                "xk": xk[b_],
                "xv": xv[b_],
                "wq": pack_w(Wq, g),
                "wk": pack_w(Wk, g),
                "wv": pack_w(Wv, g),
                "bq": pack_b(bq, g),
                "bk": pack_b(bk, g),
            }
        )

    _LAST_IN_MAPS = in_maps
    res = run_bass_kernel_spmd(nc, in_maps, core_ids=list(range(NCORES)))

    out = np.empty((B, S, H * A), dtype=np.float32)
    for i in range(NCORES):
        b_, g = i // 2, i % 2
        blk = res.results[i]["out"]  # [HL, NSQ, 65, 512]
        o = blk[:, :, :A, :] / blk[:, :, A : A + 1, :]  # divide by denom row
        o = o + bv[g * HL : (g + 1) * HL][:, None, :, None]  # bias post-divide
        # [HL, NSQ, A, 512] -> [S, HL*A]
        out[b_, :, g * HL * A : (g + 1) * HL * A] = (
            o.transpose(1, 3, 0, 2).reshape(S, HL * A)
        )
    return out
